# revision 1
# baseline (speedup 1.0000x reference)
"""HAN metapath-attention kernel for 8 Trainium2 NeuronCores (Bass/Tile).

Reference computation (B=512, P=64, K=8, D=512, T=50000):
    ref_embs = inputs[nbr_batch, nbr_job]            # [B,P,K,D] gather
    raw_s    = einsum('bpd,bpkd->bpk', inputs, ref_embs)
    sim      = softmax(where(mask, raw_s, -1e9)) * mask
    out      = concat([inputs, einsum('bpk,bpkt->bpt', sim, title[nbr_title])], -1)
    returns [B*P, 2D] f32

Sharding: data-parallel over flat rows r = b*P + p; core c owns rows
[c*4096, (c+1)*4096). The flattened `inputs` gather table is replicated to
every core's HBM; `title_emb_mat` is compacted per core (only referenced
rows; always <= 32768 distinct so local ids fit int16). No cross-core
traffic is needed.

Sparsity: mask ~ Bernoulli(1/2) makes half the neighbor slots contribute
EXACTLY zero (sim is exactly 0 there; masked slots' exp underflows to exact
0.0 in the softmax denominator), so the kernel only gathers valid slots.
Focals are bucketed by valid-count v into widths W in {2,4,6,8}; each
128-focal tile is uniform-W with a +0/-1e9 mask bias covering the v < W
remainder slots. v=0 focals (graph half exactly 0) never touch the device.
This is exact, not an approximation, and roughly halves the random-row
gather traffic that bounds this kernel (~230 GB/s/core effective for random
2KB rows from HBM).

Device work per 128-focal tile: one batched int16 dma_gather each for the
neighbor-job rows and title rows (superblocked, <= 1024 indices/op), a
sequential HWDGE load of the (host pre-permuted) focal rows, fused
scalar_tensor_tensor product+row-reduce for the dots, a free-dim masked
softmax, a fused mult+add chain for the weighted title sum, and one
[128,512] store of the graph half. The focal half of the output is
host-assembled (it is literally the input rows); device output rows are
unpermuted on the host.
"""

import sys
import time

if "/opt/trn_rl_repo" not in sys.path:
    sys.path.insert(0, "/opt/trn_rl_repo")

import numpy as np

import concourse.bacc as bacc
import concourse.bass as bass
import concourse.tile as tile
from concourse import mybir

B, P, K, D, T = 512, 64, 8, 512, 50000
NCORES = 8
R = B * P // NCORES  # 4096 focal rows per core
F32 = mybir.dt.float32
I16 = mybir.dt.int16
WIDTHS = (2, 4, 6, 8)


# ------------------------------------------------------------------ layout --


def _layout(tiles_w):
    """Emission-order op layout shared by builder and host prep.

    Each op is one superblock: nt tiles of width W, with one J gather and one
    T gather (128*nt*W indices each), a sequential focal load, nt compute
    tiles, and nt output stores. Returns (ops, idx_cols, mask_cols, rows).
    """
    ops = []
    col = 0
    mcol = 0
    rowbase = 0
    for W in WIDTHS:
        SB = max(1, 8 // W)
        t = 0
        while t < tiles_w.get(W, 0):
            nt = min(SB, tiles_w[W] - t)
            jn = 128 * nt * W
            op = dict(W=W, nt=nt, jcol=col, mcol=mcol, rowbase=rowbase)
            col += jn // 16
            op["tcol"] = col
            col += jn // 16
            ops.append(op)
            mcol += nt * W
            rowbase += nt * 128
            t += nt
    return ops, col, mcol, rowbase


# ----------------------------------------------------------------- builder --


def _build_program(plan, niter=1):
    """plan = (tiles_w item tuple, u_pad). niter>1 wraps the pass in a For_i
    loop (bench-only; makes device time dominate one execution)."""
    tiles_w = dict(plan[0])
    u_pad = plan[1]
    ops, idx_cols, mask_cols, total_rows = _layout(tiles_w)
    assert total_rows > 0

    nc = bacc.Bacc("TRN2", target_bir_lowering=False, debug=False)
    emb = nc.dram_tensor("emb", [B * P, D], F32, kind="ExternalInput")
    title = nc.dram_tensor("title", [u_pad, D], F32, kind="ExternalInput")
    # bucket-ordered focal rows (host pre-permuted): row rb + t*128 + p is the
    # focal embedding of (tile t, partition p) -> sequential HWDGE loads
    focal = nc.dram_tensor("focal", [total_rows, D], F32, kind="ExternalInput")
    gidx16 = nc.dram_tensor("gidx16", [128, idx_cols], I16, kind="ExternalInput")
    maskf = nc.dram_tensor("maskf", [128, mask_cols], F32, kind="ExternalInput")
    maskb = nc.dram_tensor("maskb", [128, mask_cols], F32, kind="ExternalInput")
    outg = nc.dram_tensor("outg", [total_rows, D], F32, kind="ExternalOutput")

    with tile.TileContext(nc) as tc:
        with (
            tc.tile_pool(name="idxp", bufs=1) as idxp,
            tc.tile_pool(name="fp", bufs=2) as fp,
            tc.tile_pool(name="jp", bufs=2) as jp,
            tc.tile_pool(name="tp", bufs=2) as tp,
            tc.tile_pool(name="wp", bufs=3) as wp,
            tc.tile_pool(name="sp", bufs=4) as sp,
        ):
            gx = idxp.tile([128, idx_cols], I16)
            mf = idxp.tile([128, mask_cols], F32)
            mb = idxp.tile([128, mask_cols], F32)
            nc.sync.dma_start(out=gx[:], in_=gidx16[:])
            nc.sync.dma_start(out=mf[:], in_=maskf[:])
            nc.sync.dma_start(out=mb[:], in_=maskb[:])

            import contextlib

            loop_ctx = (
                tc.For_i(0, niter, 1) if niter > 1 else contextlib.nullcontext()
            )
            with loop_ctx:
                for op in ops:
                    W, nt = op["W"], op["nt"]
                    jn = 128 * nt * W
                    rb0 = op["rowbase"]
                    Fs = fp.tile([128, 8, D], F32, tag="F")
                    nc.sync.dma_start(
                        out=Fs[:, :nt, :],
                        in_=focal[rb0 : rb0 + nt * 128, :].rearrange(
                            "(t p) d -> p t d", p=128
                        ),
                    )
                    Js = jp.tile([128, 8, D], F32, tag="J")
                    nc.gpsimd.dma_gather(
                        Js[:, : nt * W, :],
                        emb[:],
                        gx[:, op["jcol"] : op["jcol"] + jn // 16],
                        jn,
                        jn,
                        D,
                    )
                    Ts = tp.tile([128, 8, D], F32, tag="T")
                    nc.gpsimd.dma_gather(
                        Ts[:, : nt * W, :],
                        title[:],
                        gx[:, op["tcol"] : op["tcol"] + jn // 16],
                        jn,
                        jn,
                        D,
                    )
                    for t in range(nt):
                        mc = op["mcol"] + t * W
                        # dots[:, k] = sum_d F * J_k (fused product+row-reduce)
                        dots = sp.tile([128, 8], F32, tag="dots")
                        prod = wp.tile([128, D], F32, tag="prod")
                        for k in range(W):
                            nc.vector.scalar_tensor_tensor(
                                out=prod[:],
                                in0=Fs[:, t, :],
                                scalar=1.0,
                                in1=Js[:, t * W + k, :],
                                op0=mybir.AluOpType.mult,
                                op1=mybir.AluOpType.mult,
                                accum_out=dots[:, k : k + 1],
                            )
                        # masked logits = dots + (0 | -1e9)
                        logits = sp.tile([128, 8], F32, tag="logits")
                        nc.vector.tensor_tensor(
                            out=logits[:, :W],
                            in0=dots[:, :W],
                            in1=mb[:, mc : mc + W],
                            op=mybir.AluOpType.add,
                        )
                        negM = sp.tile([128, 1], F32, tag="negM")
                        nc.vector.tensor_reduce(
                            out=negM[:],
                            in_=logits[:, :W],
                            axis=mybir.AxisListType.X,
                            op=mybir.AluOpType.max,
                            negate=True,
                        )
                        e = sp.tile([128, 8], F32, tag="e")
                        nc.scalar.activation(
                            out=e[:, :W],
                            in_=logits[:, :W],
                            func=mybir.ActivationFunctionType.Exp,
                            bias=negM[:, 0:1],
                            scale=1.0,
                        )
                        ssum = sp.tile([128, 1], F32, tag="ssum")
                        nc.vector.tensor_reduce(
                            out=ssum[:],
                            in_=e[:, :W],
                            axis=mybir.AxisListType.X,
                            op=mybir.AluOpType.add,
                        )
                        rr = sp.tile([128, 1], F32, tag="rr")
                        nc.vector.reciprocal(out=rr[:], in_=ssum[:])
                        sim = sp.tile([128, 8], F32, tag="sim")
                        nc.vector.scalar_tensor_tensor(
                            out=sim[:, :W],
                            in0=e[:, :W],
                            scalar=rr[:, 0:1],
                            in1=mf[:, mc : mc + W],
                            op0=mybir.AluOpType.mult,
                            op1=mybir.AluOpType.mult,
                        )
                        # weighted title sum (fused mult+add chain)
                        acc = wp.tile([128, D], F32, tag="acc")
                        nc.vector.tensor_scalar_mul(
                            out=acc[:], in0=Ts[:, t * W, :], scalar1=sim[:, 0:1]
                        )
                        for k in range(1, W):
                            nc.vector.scalar_tensor_tensor(
                                out=acc[:],
                                in0=Ts[:, t * W + k, :],
                                scalar=sim[:, k : k + 1],
                                in1=acc[:],
                                op0=mybir.AluOpType.mult,
                                op1=mybir.AluOpType.add,
                            )
                        rb = op["rowbase"] + t * 128
                        nc.sync.dma_start(out=outg[rb : rb + 128, :], in_=acc[:])
    nc.finalize()
    return nc


# --------------------------------------------------------------- host prep --


def _wrap_ops(flat_lists):
    """Concat per-op flat index lists into the [128, cols] int16 idx tile.
    dma_gather consumes list l with out[p, j, :] = tbl[l[j*128+p]]; element i
    of each op's list lives at [i % 16, i // 16] of its column block,
    replicated x8 across partition groups (one per Q7 core)."""
    blocks = []
    for L in flat_lists:
        L = np.asarray(L, dtype=np.int16)
        assert len(L) % 16 == 0
        blocks.append(L.reshape(-1, 16).T)
    w = np.concatenate(blocks, axis=1)
    return np.ascontiguousarray(np.tile(w, (8, 1)))


def _sparse_host(inputs, title_emb_mat, nbr_batch, nbr_job, nbr_title, nbr_mask):
    """Bucket/compact per core. Returns None if no valid slots exist anywhere
    (output is then pure host assembly), else
    (plan, in_maps, row_focal per core, emb)."""
    inputs = np.asarray(inputs, dtype=np.float32)
    title_emb_mat = np.asarray(title_emb_mat, dtype=np.float32)
    emb = np.ascontiguousarray(inputs.reshape(B * P, D))
    jidx = (
        np.asarray(nbr_batch, dtype=np.int64) * P + np.asarray(nbr_job, dtype=np.int64)
    ).reshape(B * P, K)
    tidx = np.asarray(nbr_title, dtype=np.int64).reshape(B * P, K)
    m = np.asarray(nbr_mask, dtype=np.int64).reshape(B * P, K)

    percore = []
    for c in range(NCORES):
        rows = slice(c * R, (c + 1) * R)
        mrow = m[rows]
        v = mrow.sum(1)
        # valid slots first, ascending k among valid (keeps the fp reduce
        # order equal to the reference: adding exact zeros is an fp no-op)
        order = np.argsort(-mrow, axis=1, kind="stable")
        js = np.take_along_axis(jidx[rows], order, 1)
        tits = np.take_along_axis(tidx[rows], order, 1)
        valid_t = tits[mrow.astype(bool)[np.arange(R)[:, None], order]]
        uniq = np.unique(valid_t)
        lut = np.zeros(T, dtype=np.int64)
        lut[uniq] = np.arange(len(uniq))
        percore.append(dict(v=v, js=js, tl=lut[tits], uniq=uniq))

    nw = {
        W: max(int(((pc["v"] > W - 2) & (pc["v"] <= W)).sum()) for pc in percore)
        for W in WIDTHS
    }
    tiles_w = {W: -(-nw[W] // 128) for W in WIDTHS if nw[W] > 0}
    if not tiles_w:
        return None
    u_pad = max(512, -(-max(len(pc["uniq"]) for pc in percore) // 512) * 512)
    ops, idx_cols, mask_cols, total_rows = _layout(tiles_w)

    in_maps = []
    row_focal_all = []
    for c in range(NCORES):
        pc = percore[c]
        v, js, tl = pc["v"], pc["js"], pc["tl"]
        sel_w = {}
        for W in tiles_w:
            sel = np.where((v > W - 2) & (v <= W))[0]
            pad = tiles_w[W] * 128 - len(sel)
            sel_w[W] = np.concatenate([sel, np.full(pad, -1, dtype=np.int64)])
        flat_lists = []
        mfs = np.zeros((128, mask_cols), dtype=np.float32)
        mbs = np.full((128, mask_cols), -1e9, dtype=np.float32)
        row_focal = np.full(total_rows, -1, dtype=np.int64)
        focal_perm = np.zeros(total_rows, dtype=np.int64)
        used_w = {W: 0 for W in sel_w}
        for op in ops:
            W, nt = op["W"], op["nt"]
            sel = sel_w[W][used_w[W] : used_w[W] + nt * 128]
            used_w[W] += nt * 128
            focs = np.where(sel < 0, 0, sel)  # dummy focal -> row 0
            vv = np.where(sel < 0, 0, v[sel])  # dummy -> fully masked
            focal_perm[op["rowbase"] : op["rowbase"] + nt * 128] = focs
            jmat = np.where(np.arange(W)[None, :] < vv[:, None], js[focs, :W], 0)
            tmat = np.where(np.arange(W)[None, :] < vv[:, None], tl[focs, :W], 0)

            def mk(mat):
                return mat.reshape(nt, 128, W).transpose(0, 2, 1).reshape(-1)

            flat_lists.append(mk(jmat))
            flat_lists.append(mk(tmat))
            mvalid = (np.arange(W)[None, :] < vv[:, None]).reshape(nt, 128, W)
            for t in range(nt):
                mcol = op["mcol"] + t * W
                mfs[:, mcol : mcol + W] = mvalid[t].astype(np.float32)
                mbs[:, mcol : mcol + W] = (mvalid[t].astype(np.float32) - 1.0) * 1e9
            row_focal[op["rowbase"] : op["rowbase"] + nt * 128] = sel
        row_focal_all.append(row_focal)

        tloc = np.zeros((u_pad, D), dtype=np.float32)
        tloc[: len(pc["uniq"])] = title_emb_mat[pc["uniq"]]
        in_maps.append(
            {
                "emb": emb,
                "title": tloc,
                "focal": np.ascontiguousarray(emb[c * R + focal_perm]),
                "gidx16": _wrap_ops(flat_lists),
                "maskf": mfs,
                "maskb": mbs,
            }
        )
    plan = (tuple(sorted(tiles_w.items())), u_pad)
    return plan, in_maps, row_focal_all, emb


# ------------------------------------------------------------------ runner --

_RUNNERS = {}


class _Runner:
    """Caches the sharded jit executable for one program variant so repeated
    executions skip retracing/recompiling (adapted from
    concourse.bass2jax.run_bass_via_pjrt's multi-core branch)."""

    def __init__(self, plan, niter):
        import jax
        from jax.experimental.shard_map import shard_map
        from jax.sharding import Mesh, NamedSharding, PartitionSpec

        from concourse import mybir as _mb
        from concourse.bass2jax import (
            _bass_exec_p,
            install_neuronx_cc_hook,
            partition_id_tensor,
        )

        install_neuronx_cc_hook()
        self.jax = jax
        nc = _build_program(plan, niter)
        self.nc = nc

        in_names, out_names, out_avals = [], [], []
        partition_name = nc.partition_id_tensor.name if nc.partition_id_tensor else None
        for alloc in nc.m.functions[0].allocations:
            if not isinstance(alloc, _mb.MemoryLocationSet):
                continue
            name = alloc.memorylocations[0].name
            if alloc.kind == "ExternalInput":
                if name != partition_name:
                    in_names.append(name)
            elif alloc.kind == "ExternalOutput":
                out_names.append(name)
                out_avals.append(
                    jax.core.ShapedArray(
                        tuple(alloc.tensor_shape), _mb.dt.np(alloc.dtype)
                    )
                )

        self.in_names = in_names
        self.out_names = out_names
        self.out_avals = out_avals
        n_params = len(in_names)
        n_outs = len(out_avals)

        bind_in_names = list(in_names) + list(out_names)
        if partition_name is not None:
            bind_in_names.append(partition_name)

        def _body(*args):
            operands = list(args)
            if partition_name is not None:
                operands.append(partition_id_tensor())
            outs = _bass_exec_p.bind(
                *operands,
                out_avals=tuple(out_avals),
                in_names=tuple(bind_in_names),
                out_names=tuple(out_names),
                lowering_input_output_aliases=(),
                sim_require_finite=True,
                sim_require_nnan=True,
                nc=nc,
            )
            return tuple(outs)

        devices = jax.devices()[:NCORES]
        mesh = Mesh(np.asarray(devices), ("core",))
        self.sharding = NamedSharding(mesh, PartitionSpec("core"))
        in_specs = (PartitionSpec("core"),) * (n_params + n_outs)
        out_specs = (PartitionSpec("core"),) * n_outs
        donate = tuple(range(n_params, n_params + n_outs))
        self.fn = jax.jit(
            shard_map(
                _body,
                mesh=mesh,
                in_specs=in_specs,
                out_specs=out_specs,
                check_rep=False,
            ),
            donate_argnums=donate,
            keep_unused=True,
        )

    def place_inputs(self, in_maps):
        concat = [
            np.concatenate([np.asarray(m[name]) for m in in_maps], axis=0)
            for name in self.in_names
        ]
        return [self.jax.device_put(a, self.sharding) for a in concat]

    def make_zeros(self):
        return [
            self.jax.device_put(
                np.zeros((NCORES * av.shape[0], *av.shape[1:]), av.dtype),
                self.sharding,
            )
            for av in self.out_avals
        ]

    def run(self, dev_in, zeros):
        return self.fn(*dev_in, *zeros)


def _get_runner(plan, niter=1):
    key = (plan, niter)
    if key not in _RUNNERS:
        _RUNNERS[key] = _Runner(plan, niter)
    return _RUNNERS[key]


# -------------------------------------------------------------- public API --


def kernel(inputs, title_emb_mat, nbr_batch, nbr_job, nbr_title, nbr_mask):
    inputs = np.asarray(inputs, dtype=np.float32)
    emb = np.ascontiguousarray(inputs.reshape(B * P, D))
    prep = _sparse_host(
        inputs, title_emb_mat, nbr_batch, nbr_job, nbr_title, nbr_mask
    )
    out = np.zeros((B * P, 2 * D), dtype=np.float32)
    out[:, :D] = emb  # focal half of the concat is literally the input rows
    if prep is None:  # every slot masked: graph half is exactly zero
        return out
    plan, in_maps, row_focal_all, _ = prep

    runner = _get_runner(plan, 1)
    dev_in = runner.place_inputs(in_maps)
    outs = runner.run(dev_in, runner.make_zeros())
    outg_full = np.asarray(outs[runner.out_names.index("outg")])
    total_rows = outg_full.shape[0] // NCORES
    for c in range(NCORES):
        outg = outg_full[c * total_rows : (c + 1) * total_rows]
        rf = row_focal_all[c]
        valid = rf >= 0
        out[c * R + rf[valid], D:] = outg[valid]
    return out


def bench(in_maps, plan, niters=(65, 257), reps=8):
    """Per-pass device time via on-device For_i iteration scaling; min-stat
    over reps cancels most of the axon RPC jitter."""
    results = {}
    for ni in niters:
        runner = _get_runner(plan, ni)
        dev_in = runner.place_inputs(in_maps)
        zeros = [runner.make_zeros() for _ in range(reps + 1)]
        out = runner.run(dev_in, zeros[0])
        for o in out:
            o.block_until_ready()
        ts = []
        for r in range(reps):
            t0 = time.perf_counter()
            outs = runner.run(dev_in, zeros[r + 1])
            for o in outs:
                o.block_until_ready()
            ts.append(time.perf_counter() - t0)
        results[ni] = min(ts)
        print(
            f"  niter={ni}: min {min(ts) * 1e3:.3f} ms  "
            f"(all: {', '.join(f'{t * 1e3:.2f}' for t in sorted(ts))})",
            flush=True,
        )
    ni_lo, ni_hi = min(niters), max(niters)
    per_pass = (results[ni_hi] - results[ni_lo]) / (ni_hi - ni_lo)
    return per_pass * 1e9, results



# revision 2
# speedup vs baseline: 1.2632x; 1.2632x over previous
"""HAN metapath-attention kernel for 8 Trainium2 NeuronCores (Bass/Tile).

Reference computation (B=512, P=64, K=8, D=512, T=50000):
    ref_embs = inputs[nbr_batch, nbr_job]            # [B,P,K,D] gather
    raw_s    = einsum('bpd,bpkd->bpk', inputs, ref_embs)
    sim      = softmax(where(mask, raw_s, -1e9)) * mask
    out      = concat([inputs, einsum('bpk,bpkt->bpt', sim, title[nbr_title])], -1)
    returns [B*P, 2D] f32

Sharding: data-parallel over flat rows r = b*P + p; core c owns rows
[c*4096, (c+1)*4096). The flattened `inputs` gather table is replicated to
every core's HBM; `title_emb_mat` is compacted per core (only referenced
rows; always <= 32767 distinct so local ids fit int16). No cross-core
traffic is needed.

Sparsity: mask ~ Bernoulli(1/2) makes half the neighbor slots contribute
EXACTLY zero, so the kernel only gathers valid slots. Focals are bucketed
by exact valid-count v = W in {1..8}; each 128-focal tile is uniform-W so
no per-slot masking waste remains (only <128 round-up pad focals per
bucket). v=0 focals (graph half exactly 0) never touch the device.

Precision: all embedding tables (focal rows, neighbor-job rows, title
rows) are bf16 on device, halving the random-row gather traffic that
bounds this kernel. Dot-product logits accumulate in f32 (stt accum_out);
softmax runs in f32; the weighted title accumulation and output store are
bf16. Emulated end-to-end rel-err vs the f32 reference is 5.8e-3, within
the 2e-2 gate.

Device work per 128-focal tile: one batched int16 dma_gather each for the
neighbor-job rows (SWDGE queue 0) and title rows (queue 1), superblocked
<= 1024 indices/op, a sequential HWDGE load of the (host pre-permuted)
focal rows, fused bf16 product+f32-row-reduce for the dots, a free-dim
masked softmax, a fused bf16 mult+add chain for the weighted title sum,
and one [128,512] bf16 store of the graph half. The focal half of the
output is host-assembled (it is literally the input rows); device output
rows are unpermuted on the host.
"""

import sys
import time

if "/opt/trn_rl_repo" not in sys.path:
    sys.path.insert(0, "/opt/trn_rl_repo")

import numpy as np
import ml_dtypes

import concourse.bacc as bacc
import concourse.bass as bass
import concourse.tile as tile
from concourse import mybir

B, P, K, D, T = 512, 64, 8, 512, 50000
NCORES = 8
R = B * P // NCORES  # 4096 focal rows per core
F32 = mybir.dt.float32
BF16 = mybir.dt.bfloat16
I16 = mybir.dt.int16
NPBF16 = ml_dtypes.bfloat16
WIDTHS = (1, 2, 3, 4, 5, 6, 7, 8)


# ------------------------------------------------------------------ layout --


def _layout(tiles_w):
    """Emission-order op layout shared by builder and host prep.

    Each op is one superblock: nt tiles of width W, with one J gather and one
    T gather (128*nt*W indices each), a sequential focal load, nt compute
    tiles, and nt output stores. Returns (ops, idx_cols, mask_cols, rows).
    """
    ops = []
    col = 0
    mcol = 0
    rowbase = 0
    for W in WIDTHS:
        SB = max(1, 8 // W)
        t = 0
        while t < tiles_w.get(W, 0):
            nt = min(SB, tiles_w[W] - t)
            jn = 128 * nt * W
            op = dict(W=W, nt=nt, jcol=col, mcol=mcol, rowbase=rowbase)
            col += jn // 16
            op["tcol"] = col
            col += jn // 16
            ops.append(op)
            mcol += nt * W
            rowbase += nt * 128
            t += nt
    return ops, col, mcol, rowbase


# ----------------------------------------------------------------- builder --


def _build_program(plan, niter=1):
    """plan = (tiles_w item tuple, u_pad). niter>1 wraps the pass in a For_i
    loop (bench-only; makes device time dominate one execution)."""
    tiles_w = dict(plan[0])
    u_pad = plan[1]
    ops, idx_cols, mask_cols, total_rows = _layout(tiles_w)
    assert total_rows > 0

    nc = bacc.Bacc(
        "TRN2", target_bir_lowering=False, debug=False, num_swdge_queues=2
    )
    emb = nc.dram_tensor("emb", [B * P, D], BF16, kind="ExternalInput")
    title = nc.dram_tensor("title", [u_pad, D], BF16, kind="ExternalInput")
    # bucket-ordered focal rows (host pre-permuted): row rb + t*128 + p is the
    # focal embedding of (tile t, partition p) -> sequential HWDGE loads
    focal = nc.dram_tensor("focal", [total_rows, D], BF16, kind="ExternalInput")
    gidx16 = nc.dram_tensor("gidx16", [128, idx_cols], I16, kind="ExternalInput")
    maskf = nc.dram_tensor("maskf", [128, mask_cols], F32, kind="ExternalInput")
    maskb = nc.dram_tensor("maskb", [128, mask_cols], F32, kind="ExternalInput")
    outg = nc.dram_tensor("outg", [total_rows, D], BF16, kind="ExternalOutput")

    with tile.TileContext(nc) as tc:
        with (
            tc.tile_pool(name="idxp", bufs=1) as idxp,
            tc.tile_pool(name="fp", bufs=2) as fp,
            tc.tile_pool(name="jp", bufs=2) as jp,
            tc.tile_pool(name="tp", bufs=2) as tp,
            tc.tile_pool(name="wp", bufs=3) as wp,
            tc.tile_pool(name="sp", bufs=4) as sp,
        ):
            gx = idxp.tile([128, idx_cols], I16)
            mf = idxp.tile([128, mask_cols], F32)
            mb = idxp.tile([128, mask_cols], F32)
            nc.sync.dma_start(out=gx[:], in_=gidx16[:])
            nc.sync.dma_start(out=mf[:], in_=maskf[:])
            nc.sync.dma_start(out=mb[:], in_=maskb[:])

            import contextlib

            loop_ctx = (
                tc.For_i(0, niter, 1) if niter > 1 else contextlib.nullcontext()
            )
            with loop_ctx:
                for op in ops:
                    W, nt = op["W"], op["nt"]
                    jn = 128 * nt * W
                    rb0 = op["rowbase"]
                    Fs = fp.tile([128, 8, D], BF16, tag="F")
                    nc.sync.dma_start(
                        out=Fs[:, :nt, :],
                        in_=focal[rb0 : rb0 + nt * 128, :].rearrange(
                            "(t p) d -> p t d", p=128
                        ),
                    )
                    Js = jp.tile([128, 8, D], BF16, tag="J")
                    nc.gpsimd.dma_gather(
                        Js[:, : nt * W, :],
                        emb[:],
                        gx[:, op["jcol"] : op["jcol"] + jn // 16],
                        jn,
                        jn,
                        D,
                        queue_num=0,
                    )
                    Ts = tp.tile([128, 8, D], BF16, tag="T")
                    nc.gpsimd.dma_gather(
                        Ts[:, : nt * W, :],
                        title[:],
                        gx[:, op["tcol"] : op["tcol"] + jn // 16],
                        jn,
                        jn,
                        D,
                        queue_num=1,
                    )
                    for t in range(nt):
                        mc = op["mcol"] + t * W
                        acc = wp.tile([128, D], BF16, tag="acc")
                        if W == 1:
                            # single valid slot: sim == mask (1 valid, 0 pad)
                            nc.vector.tensor_scalar_mul(
                                out=acc[:],
                                in0=Ts[:, t, :],
                                scalar1=mf[:, mc : mc + 1],
                            )
                            rb = op["rowbase"] + t * 128
                            nc.sync.dma_start(
                                out=outg[rb : rb + 128, :], in_=acc[:]
                            )
                            continue
                        # dots[:, k] = sum_d F * J_k (fused product+row-reduce)
                        dots = sp.tile([128, 8], F32, tag="dots")
                        prod = wp.tile([128, D], BF16, tag="prod")
                        for k in range(W):
                            nc.vector.scalar_tensor_tensor(
                                out=prod[:],
                                in0=Fs[:, t, :],
                                scalar=1.0,
                                in1=Js[:, t * W + k, :],
                                op0=mybir.AluOpType.mult,
                                op1=mybir.AluOpType.mult,
                                accum_out=dots[:, k : k + 1],
                            )
                        # masked logits = dots + (0 | -1e9)
                        logits = sp.tile([128, 8], F32, tag="logits")
                        nc.vector.tensor_tensor(
                            out=logits[:, :W],
                            in0=dots[:, :W],
                            in1=mb[:, mc : mc + W],
                            op=mybir.AluOpType.add,
                        )
                        negM = sp.tile([128, 1], F32, tag="negM")
                        nc.vector.tensor_reduce(
                            out=negM[:],
                            in_=logits[:, :W],
                            axis=mybir.AxisListType.X,
                            op=mybir.AluOpType.max,
                            negate=True,
                        )
                        e = sp.tile([128, 8], F32, tag="e")
                        nc.scalar.activation(
                            out=e[:, :W],
                            in_=logits[:, :W],
                            func=mybir.ActivationFunctionType.Exp,
                            bias=negM[:, 0:1],
                            scale=1.0,
                        )
                        ssum = sp.tile([128, 1], F32, tag="ssum")
                        nc.vector.tensor_reduce(
                            out=ssum[:],
                            in_=e[:, :W],
                            axis=mybir.AxisListType.X,
                            op=mybir.AluOpType.add,
                        )
                        rr = sp.tile([128, 1], F32, tag="rr")
                        nc.vector.reciprocal(out=rr[:], in_=ssum[:])
                        sim = sp.tile([128, 8], F32, tag="sim")
                        nc.vector.scalar_tensor_tensor(
                            out=sim[:, :W],
                            in0=e[:, :W],
                            scalar=rr[:, 0:1],
                            in1=mf[:, mc : mc + W],
                            op0=mybir.AluOpType.mult,
                            op1=mybir.AluOpType.mult,
                        )
                        # weighted title sum (fused mult+add chain)
                        nc.vector.tensor_scalar_mul(
                            out=acc[:], in0=Ts[:, t * W, :], scalar1=sim[:, 0:1]
                        )
                        for k in range(1, W):
                            nc.vector.scalar_tensor_tensor(
                                out=acc[:],
                                in0=Ts[:, t * W + k, :],
                                scalar=sim[:, k : k + 1],
                                in1=acc[:],
                                op0=mybir.AluOpType.mult,
                                op1=mybir.AluOpType.add,
                            )
                        rb = op["rowbase"] + t * 128
                        nc.sync.dma_start(out=outg[rb : rb + 128, :], in_=acc[:])
    nc.finalize()
    return nc


# --------------------------------------------------------------- host prep --


def _wrap_ops(flat_lists):
    """Concat per-op flat index lists into the [128, cols] int16 idx tile.
    dma_gather consumes list l with out[p, j, :] = tbl[l[j*128+p]]; element i
    of each op's list lives at [i % 16, i // 16] of its column block,
    replicated x8 across partition groups (one per Q7 core)."""
    blocks = []
    for L in flat_lists:
        L = np.asarray(L, dtype=np.int16)
        assert len(L) % 16 == 0
        blocks.append(L.reshape(-1, 16).T)
    w = np.concatenate(blocks, axis=1)
    return np.ascontiguousarray(np.tile(w, (8, 1)))


def _sparse_host(inputs, title_emb_mat, nbr_batch, nbr_job, nbr_title, nbr_mask):
    """Bucket/compact per core. Returns None if no valid slots exist anywhere
    (output is then pure host assembly), else
    (plan, in_maps, row_focal per core, emb)."""
    inputs = np.asarray(inputs, dtype=np.float32)
    title_emb_mat = np.asarray(title_emb_mat, dtype=np.float32)
    emb = np.ascontiguousarray(inputs.reshape(B * P, D))
    emb16 = emb.astype(NPBF16)
    jidx = (
        np.asarray(nbr_batch, dtype=np.int64) * P + np.asarray(nbr_job, dtype=np.int64)
    ).reshape(B * P, K)
    tidx = np.asarray(nbr_title, dtype=np.int64).reshape(B * P, K)
    m = np.asarray(nbr_mask, dtype=np.int64).reshape(B * P, K)

    percore = []
    for c in range(NCORES):
        rows = slice(c * R, (c + 1) * R)
        mrow = m[rows]
        v = mrow.sum(1)
        # valid slots first, ascending k among valid (keeps the fp reduce
        # order equal to the reference: adding exact zeros is an fp no-op)
        order = np.argsort(-mrow, axis=1, kind="stable")
        js = np.take_along_axis(jidx[rows], order, 1)
        tits = np.take_along_axis(tidx[rows], order, 1)
        valid_t = tits[mrow.astype(bool)[np.arange(R)[:, None], order]]
        uniq = np.unique(valid_t)
        lut = np.zeros(T, dtype=np.int64)
        lut[uniq] = np.arange(len(uniq))
        percore.append(dict(v=v, js=js, tl=lut[tits], uniq=uniq))

    nw = {
        W: max(int((pc["v"] == W).sum()) for pc in percore) for W in WIDTHS
    }
    tiles_w = {W: -(-nw[W] // 128) for W in WIDTHS if nw[W] > 0}
    if not tiles_w:
        return None
    u_pad = max(512, -(-max(len(pc["uniq"]) for pc in percore) // 512) * 512)
    ops, idx_cols, mask_cols, total_rows = _layout(tiles_w)

    in_maps = []
    row_focal_all = []
    for c in range(NCORES):
        pc = percore[c]
        v, js, tl = pc["v"], pc["js"], pc["tl"]
        sel_w = {}
        for W in tiles_w:
            sel = np.where(v == W)[0]
            pad = tiles_w[W] * 128 - len(sel)
            sel_w[W] = np.concatenate([sel, np.full(pad, -1, dtype=np.int64)])
        flat_lists = []
        mfs = np.zeros((128, mask_cols), dtype=np.float32)
        mbs = np.full((128, mask_cols), -1e9, dtype=np.float32)
        row_focal = np.full(total_rows, -1, dtype=np.int64)
        focal_perm = np.zeros(total_rows, dtype=np.int64)
        used_w = {W: 0 for W in sel_w}
        for op in ops:
            W, nt = op["W"], op["nt"]
            sel = sel_w[W][used_w[W] : used_w[W] + nt * 128]
            used_w[W] += nt * 128
            focs = np.where(sel < 0, 0, sel)  # dummy focal -> row 0
            vv = np.where(sel < 0, 0, v[sel])  # dummy -> fully masked
            focal_perm[op["rowbase"] : op["rowbase"] + nt * 128] = focs
            jmat = np.where(np.arange(W)[None, :] < vv[:, None], js[focs, :W], 0)
            tmat = np.where(np.arange(W)[None, :] < vv[:, None], tl[focs, :W], 0)

            def mk(mat):
                return mat.reshape(nt, 128, W).transpose(0, 2, 1).reshape(-1)

            flat_lists.append(mk(jmat))
            flat_lists.append(mk(tmat))
            mvalid = (np.arange(W)[None, :] < vv[:, None]).reshape(nt, 128, W)
            for t in range(nt):
                mcol = op["mcol"] + t * W
                mfs[:, mcol : mcol + W] = mvalid[t].astype(np.float32)
                mbs[:, mcol : mcol + W] = (mvalid[t].astype(np.float32) - 1.0) * 1e9
            row_focal[op["rowbase"] : op["rowbase"] + nt * 128] = sel
        row_focal_all.append(row_focal)

        tloc = np.zeros((u_pad, D), dtype=NPBF16)
        tloc[: len(pc["uniq"])] = title_emb_mat[pc["uniq"]].astype(NPBF16)
        in_maps.append(
            {
                "emb": emb16,
                "title": tloc,
                "focal": np.ascontiguousarray(emb16[c * R + focal_perm]),
                "gidx16": _wrap_ops(flat_lists),
                "maskf": mfs,
                "maskb": mbs,
            }
        )
    plan = (tuple(sorted(tiles_w.items())), u_pad)
    return plan, in_maps, row_focal_all, emb


# ------------------------------------------------------------------ runner --

_RUNNERS = {}


class _Runner:
    """Caches the sharded jit executable for one program variant so repeated
    executions skip retracing/recompiling (adapted from
    concourse.bass2jax.run_bass_via_pjrt's multi-core branch)."""

    def __init__(self, plan, niter):
        import jax
        from jax.experimental.shard_map import shard_map
        from jax.sharding import Mesh, NamedSharding, PartitionSpec

        from concourse import mybir as _mb
        from concourse.bass2jax import (
            _bass_exec_p,
            install_neuronx_cc_hook,
            partition_id_tensor,
        )

        install_neuronx_cc_hook()
        self.jax = jax
        nc = _build_program(plan, niter)
        self.nc = nc

        in_names, out_names, out_avals = [], [], []
        partition_name = nc.partition_id_tensor.name if nc.partition_id_tensor else None
        for alloc in nc.m.functions[0].allocations:
            if not isinstance(alloc, _mb.MemoryLocationSet):
                continue
            name = alloc.memorylocations[0].name
            if alloc.kind == "ExternalInput":
                if name != partition_name:
                    in_names.append(name)
            elif alloc.kind == "ExternalOutput":
                out_names.append(name)
                out_avals.append(
                    jax.core.ShapedArray(
                        tuple(alloc.tensor_shape), _mb.dt.np(alloc.dtype)
                    )
                )

        self.in_names = in_names
        self.out_names = out_names
        self.out_avals = out_avals
        n_params = len(in_names)
        n_outs = len(out_avals)

        bind_in_names = list(in_names) + list(out_names)
        if partition_name is not None:
            bind_in_names.append(partition_name)

        def _body(*args):
            operands = list(args)
            if partition_name is not None:
                operands.append(partition_id_tensor())
            outs = _bass_exec_p.bind(
                *operands,
                out_avals=tuple(out_avals),
                in_names=tuple(bind_in_names),
                out_names=tuple(out_names),
                lowering_input_output_aliases=(),
                sim_require_finite=True,
                sim_require_nnan=True,
                nc=nc,
            )
            return tuple(outs)

        devices = jax.devices()[:NCORES]
        mesh = Mesh(np.asarray(devices), ("core",))
        self.sharding = NamedSharding(mesh, PartitionSpec("core"))
        in_specs = (PartitionSpec("core"),) * (n_params + n_outs)
        out_specs = (PartitionSpec("core"),) * n_outs
        donate = tuple(range(n_params, n_params + n_outs))
        self.fn = jax.jit(
            shard_map(
                _body,
                mesh=mesh,
                in_specs=in_specs,
                out_specs=out_specs,
                check_rep=False,
            ),
            donate_argnums=donate,
            keep_unused=True,
        )

    def place_inputs(self, in_maps):
        concat = [
            np.concatenate([np.asarray(m[name]) for m in in_maps], axis=0)
            for name in self.in_names
        ]
        return [self.jax.device_put(a, self.sharding) for a in concat]

    def make_zeros(self):
        return [
            self.jax.device_put(
                np.zeros((NCORES * av.shape[0], *av.shape[1:]), av.dtype),
                self.sharding,
            )
            for av in self.out_avals
        ]

    def run(self, dev_in, zeros):
        return self.fn(*dev_in, *zeros)


def _get_runner(plan, niter=1):
    key = (plan, niter)
    if key not in _RUNNERS:
        _RUNNERS[key] = _Runner(plan, niter)
    return _RUNNERS[key]


# -------------------------------------------------------------- public API --


def kernel(inputs, title_emb_mat, nbr_batch, nbr_job, nbr_title, nbr_mask):
    inputs = np.asarray(inputs, dtype=np.float32)
    emb = np.ascontiguousarray(inputs.reshape(B * P, D))
    prep = _sparse_host(
        inputs, title_emb_mat, nbr_batch, nbr_job, nbr_title, nbr_mask
    )
    out = np.zeros((B * P, 2 * D), dtype=np.float32)
    out[:, :D] = emb  # focal half of the concat is literally the input rows
    if prep is None:  # every slot masked: graph half is exactly zero
        return out
    plan, in_maps, row_focal_all, _ = prep

    runner = _get_runner(plan, 1)
    dev_in = runner.place_inputs(in_maps)
    outs = runner.run(dev_in, runner.make_zeros())
    outg_full = np.asarray(outs[runner.out_names.index("outg")]).astype(np.float32)
    total_rows = outg_full.shape[0] // NCORES
    for c in range(NCORES):
        outg = outg_full[c * total_rows : (c + 1) * total_rows]
        rf = row_focal_all[c]
        valid = rf >= 0
        out[c * R + rf[valid], D:] = outg[valid]
    return out


def bench(in_maps, plan, niters=(65, 257), reps=8):
    """Per-pass device time via on-device For_i iteration scaling; min-stat
    over reps cancels most of the axon RPC jitter."""
    results = {}
    for ni in niters:
        runner = _get_runner(plan, ni)
        dev_in = runner.place_inputs(in_maps)
        zeros = [runner.make_zeros() for _ in range(reps + 1)]
        out = runner.run(dev_in, zeros[0])
        for o in out:
            o.block_until_ready()
        ts = []
        for r in range(reps):
            t0 = time.perf_counter()
            outs = runner.run(dev_in, zeros[r + 1])
            for o in outs:
                o.block_until_ready()
            ts.append(time.perf_counter() - t0)
        results[ni] = min(ts)
        print(
            f"  niter={ni}: min {min(ts) * 1e3:.3f} ms  "
            f"(all: {', '.join(f'{t * 1e3:.2f}' for t in sorted(ts))})",
            flush=True,
        )
    ni_lo, ni_hi = min(niters), max(niters)
    per_pass = (results[ni_hi] - results[ni_lo]) / (ni_hi - ni_lo)
    return per_pass * 1e9, results


# revision 27
# speedup vs baseline: 1.8023x; 1.4267x over previous
"""HAN metapath-attention kernel for 8 Trainium2 NeuronCores (Bass/Tile).

Reference computation (B=512, P=64, K=8, D=512, T=50000):
    ref_embs = inputs[nbr_batch, nbr_job]            # [B,P,K,D] gather
    raw_s    = einsum('bpd,bpkd->bpk', inputs, ref_embs)
    sim      = softmax(where(mask, raw_s, -1e9)) * mask
    out      = concat([inputs, einsum('bpk,bpkt->bpt', sim, title[nbr_title])], -1)
    returns [B*P, 2D] f32

Sharding: data-parallel over flat rows r = b*P + p; core c owns rows
[c*4096, (c+1)*4096). The flattened `inputs` gather table is replicated to
every core's HBM; `title_emb_mat` is compacted per core (only referenced
rows; always <= 32767 distinct so local ids fit int16). No cross-core
traffic is needed.

Sparsity: mask ~ Bernoulli(1/2) makes half the neighbor slots contribute
EXACTLY zero, so the kernel only gathers valid slots. Focals are bucketed
by exact valid-count v = W in {1..8}; each 128-focal tile is uniform-W so
no per-slot masking waste remains (only <128 round-up pad focals per
bucket). v=0 focals (graph half exactly 0) never touch the device.

Precision: all embedding tables (focal rows, neighbor-job rows, title
rows) are bf16 on device, halving the random-row gather traffic that
bounds this kernel. Dot-product logits accumulate in f32 (stt accum_out);
softmax runs in f32; the weighted title accumulation and output store are
bf16. Emulated end-to-end rel-err vs the f32 reference is 5.8e-3, within
the 2e-2 gate.

Device work per 128-focal tile: one batched int16 dma_gather each for the
neighbor-job rows (SWDGE queue 0) and title rows (queue 1), superblocked
<= 1024 indices/op, a sequential HWDGE load of the (host pre-permuted)
focal rows, fused bf16 product+f32-row-reduce for the dots, a free-dim
masked softmax, a fused bf16 mult+add chain for the weighted title sum,
and one [128,512] bf16 store of the graph half. The focal half of the
output is host-assembled (it is literally the input rows); device output
rows are unpermuted on the host.
"""

import sys
import time

if "/opt/trn_rl_repo" not in sys.path:
    sys.path.insert(0, "/opt/trn_rl_repo")

import numpy as np
import ml_dtypes

import concourse.bacc as bacc
import concourse.bass as bass
import concourse.tile as tile
from concourse import mybir

B, P, K, D, T = 512, 64, 8, 512, 50000
NCORES = 8
R = B * P // NCORES  # 4096 focal rows per core
F32 = mybir.dt.float32
BF16 = mybir.dt.bfloat16
I16 = mybir.dt.int16
NPBF16 = ml_dtypes.bfloat16
WIDTHS = (1, 2, 3, 4, 5, 6, 7, 8)

# Tunables (A/B swept on hardware; defaults = best measured combination).
CONFIG = dict(
    single_packet=False,  # dma_gather packetization mode
    nqueues=2,  # SWDGE queues: J on even, T on odd, alternating per op
    pmajor=False,  # p-major focal/out rows: 1 contiguous desc per partition
    sbmax=8,  # max slots (nt*W) per gather superblock
    gbufs=5,  # gather tile pool buffering (deep: keeps SDMA queues fed)
    sorted=True,  # relabel gather tables in first-access order (HBM locality)
    fbufs=4,  # focal load pool buffering
    nocompute=False,  # DEBUG: replace attention compute by one add (DMA floor)
    idx0=False,  # DEBUG: all gather indices -> row 0 (no HBM randomness)
    padspread=True,  # spread masked-slot gather indices across the table
    cbufs=True,  # deeper compute pools (wp=4, sp=6)
)


def _cfg_key():
    return tuple(sorted(CONFIG.items()))


# ------------------------------------------------------------------ layout --


def _layout(tiles_w):
    """Emission-order op layout shared by builder and host prep.

    Each op is one superblock: nt tiles of width W, with one J gather and one
    T gather (128*nt*W indices each), a sequential focal load, nt compute
    tiles, and nt output stores. Returns (ops, idx_cols, mask_cols, rows).
    """
    ops = []
    col = 0
    mcol = 0
    rowbase = 0
    for W in WIDTHS:
        SB = max(1, CONFIG["sbmax"] // W)
        t = 0
        while t < tiles_w.get(W, 0):
            nt = min(SB, tiles_w[W] - t)
            jn = 128 * nt * W
            op = dict(W=W, nt=nt, jcol=col, mcol=mcol, rowbase=rowbase)
            col += jn // 16
            op["tcol"] = col
            col += jn // 16
            ops.append(op)
            mcol += nt * W
            rowbase += nt * 128
            t += nt
    return ops, col, mcol, rowbase


# ----------------------------------------------------------------- builder --


def _build_program(plan, niter=1):
    """plan = (tiles_w item tuple, u_pad). niter>1 wraps the pass in a For_i
    loop (bench-only; makes device time dominate one execution)."""
    tiles_w = dict(plan[0])
    u_pad = plan[1]
    assert plan[2] == _cfg_key(), "plan built under a different CONFIG"
    ops, idx_cols, mask_cols, total_rows = _layout(tiles_w)
    assert total_rows > 0
    SBM = CONFIG["sbmax"]
    NQ = CONFIG["nqueues"]
    SP_ = CONFIG["single_packet"]
    PMAJ = CONFIG["pmajor"]

    nc = bacc.Bacc(
        "TRN2", target_bir_lowering=False, debug=False,
        num_swdge_queues=max(2, NQ),
    )
    j_rows = plan[3] if len(plan) > 3 else B * P
    emb = nc.dram_tensor("emb", [j_rows, D], BF16, kind="ExternalInput")
    title = nc.dram_tensor("title", [u_pad, D], BF16, kind="ExternalInput")
    # bucket-ordered focal rows (host pre-permuted): row rb + t*128 + p is the
    # focal embedding of (tile t, partition p) -> sequential HWDGE loads
    focal = nc.dram_tensor("focal", [total_rows, D], BF16, kind="ExternalInput")
    gidx16 = nc.dram_tensor("gidx16", [128, idx_cols], I16, kind="ExternalInput")
    maskf = nc.dram_tensor("maskf", [128, mask_cols], F32, kind="ExternalInput")
    maskb = nc.dram_tensor("maskb", [128, mask_cols], F32, kind="ExternalInput")
    outg = nc.dram_tensor("outg", [total_rows, D], BF16, kind="ExternalOutput")

    with tile.TileContext(nc) as tc:
        GB = CONFIG["gbufs"]
        with (
            tc.tile_pool(name="idxp", bufs=1) as idxp,
            tc.tile_pool(name="fp", bufs=CONFIG["fbufs"]) as fp,
            tc.tile_pool(name="jp", bufs=GB) as jp,
            tc.tile_pool(name="tp", bufs=GB) as tp,
            tc.tile_pool(name="wp", bufs=4 if CONFIG["cbufs"] else 3) as wp,
            tc.tile_pool(name="op_", bufs=2) as op_,
            tc.tile_pool(name="sp", bufs=6 if CONFIG["cbufs"] else 4) as sp,
        ):
            gx = idxp.tile([128, idx_cols], I16)
            mf = idxp.tile([128, mask_cols], F32)
            mb = idxp.tile([128, mask_cols], F32)
            nc.sync.dma_start(out=gx[:], in_=gidx16[:])
            nc.sync.dma_start(out=mf[:], in_=maskf[:])
            nc.sync.dma_start(out=mb[:], in_=maskb[:])

            import contextlib

            loop_ctx = (
                tc.For_i(0, niter, 1) if niter > 1 else contextlib.nullcontext()
            )
            with loop_ctx:
                for iop, op in enumerate(ops):
                    W, nt = op["W"], op["nt"]
                    jn = 128 * nt * W
                    rb0 = op["rowbase"]
                    Fs = fp.tile([128, SBM, D], BF16, tag="F")
                    if PMAJ:
                        fsrc = focal[rb0 : rb0 + nt * 128, :].rearrange(
                            "(p t) d -> p t d", t=nt
                        )
                    else:
                        fsrc = focal[rb0 : rb0 + nt * 128, :].rearrange(
                            "(t p) d -> p t d", p=128
                        )
                    nc.sync.dma_start(out=Fs[:, :nt, :], in_=fsrc)
                    jq = (2 * iop) % NQ
                    tq = (2 * iop + 1) % NQ
                    Js = jp.tile([128, SBM, D], BF16, tag="J")
                    nc.gpsimd.dma_gather(
                        Js[:, : nt * W, :],
                        emb[:],
                        gx[:, op["jcol"] : op["jcol"] + jn // 16],
                        jn,
                        jn,
                        D,
                        queue_num=jq,
                        single_packet=SP_,
                    )
                    Ts = tp.tile([128, SBM, D], BF16, tag="T")
                    nc.gpsimd.dma_gather(
                        Ts[:, : nt * W, :],
                        title[:],
                        gx[:, op["tcol"] : op["tcol"] + jn // 16],
                        jn,
                        jn,
                        D,
                        queue_num=tq,
                        single_packet=SP_,
                    )
                    Os = (
                        op_.tile([128, SBM, D], BF16, tag="Os", name="Os")
                        if PMAJ
                        else None
                    )
                    for t in range(nt):
                        mc = op["mcol"] + t * W
                        if PMAJ:
                            acc = Os[:, t, :]
                        else:
                            acc_t = wp.tile([128, D], BF16, tag="acc", name="acc")
                            acc = acc_t[:]
                        if CONFIG["nocompute"]:
                            # keep all three loads live via two adds, no attention
                            nc.vector.scalar_tensor_tensor(
                                out=acc,
                                in0=Fs[:, t, :],
                                scalar=1.0,
                                in1=Js[:, t * W, :],
                                op0=mybir.AluOpType.mult,
                                op1=mybir.AluOpType.add,
                            )
                            nc.vector.tensor_tensor(
                                out=acc,
                                in0=acc,
                                in1=Ts[:, t * W, :],
                                op=mybir.AluOpType.add,
                            )
                            if not PMAJ:
                                rb = op["rowbase"] + t * 128
                                nc.sync.dma_start(
                                    out=outg[rb : rb + 128, :], in_=acc
                                )
                            continue
                        if W == 1:
                            # single valid slot: sim == mask (1 valid, 0 pad)
                            nc.vector.tensor_scalar_mul(
                                out=acc,
                                in0=Ts[:, t, :],
                                scalar1=mf[:, mc : mc + 1],
                            )
                            if not PMAJ:
                                rb = op["rowbase"] + t * 128
                                nc.sync.dma_start(
                                    out=outg[rb : rb + 128, :], in_=acc
                                )
                            continue
                        # dots[:, k] = sum_d F * J_k (fused product+row-reduce)
                        dots = sp.tile([128, 8], F32, tag="dots")
                        prod = wp.tile([128, D], BF16, tag="prod")
                        for k in range(W):
                            nc.vector.scalar_tensor_tensor(
                                out=prod[:],
                                in0=Fs[:, t, :],
                                scalar=1.0,
                                in1=Js[:, t * W + k, :],
                                op0=mybir.AluOpType.mult,
                                op1=mybir.AluOpType.mult,
                                accum_out=dots[:, k : k + 1],
                            )
                        # masked logits = dots + (0 | -1e9)
                        logits = sp.tile([128, 8], F32, tag="logits")
                        nc.vector.tensor_tensor(
                            out=logits[:, :W],
                            in0=dots[:, :W],
                            in1=mb[:, mc : mc + W],
                            op=mybir.AluOpType.add,
                        )
                        negM = sp.tile([128, 1], F32, tag="negM")
                        nc.vector.tensor_reduce(
                            out=negM[:],
                            in_=logits[:, :W],
                            axis=mybir.AxisListType.X,
                            op=mybir.AluOpType.max,
                            negate=True,
                        )
                        e = sp.tile([128, 8], F32, tag="e")
                        nc.scalar.activation(
                            out=e[:, :W],
                            in_=logits[:, :W],
                            func=mybir.ActivationFunctionType.Exp,
                            bias=negM[:, 0:1],
                            scale=1.0,
                        )
                        ssum = sp.tile([128, 1], F32, tag="ssum")
                        nc.vector.tensor_reduce(
                            out=ssum[:],
                            in_=e[:, :W],
                            axis=mybir.AxisListType.X,
                            op=mybir.AluOpType.add,
                        )
                        rr = sp.tile([128, 1], F32, tag="rr")
                        nc.vector.reciprocal(out=rr[:], in_=ssum[:])
                        sim = sp.tile([128, 8], F32, tag="sim")
                        nc.vector.scalar_tensor_tensor(
                            out=sim[:, :W],
                            in0=e[:, :W],
                            scalar=rr[:, 0:1],
                            in1=mf[:, mc : mc + W],
                            op0=mybir.AluOpType.mult,
                            op1=mybir.AluOpType.mult,
                        )
                        # weighted title sum (fused mult+add chain)
                        nc.vector.tensor_scalar_mul(
                            out=acc, in0=Ts[:, t * W, :], scalar1=sim[:, 0:1]
                        )
                        for k in range(1, W):
                            nc.vector.scalar_tensor_tensor(
                                out=acc,
                                in0=Ts[:, t * W + k, :],
                                scalar=sim[:, k : k + 1],
                                in1=acc,
                                op0=mybir.AluOpType.mult,
                                op1=mybir.AluOpType.add,
                            )
                        if not PMAJ:
                            rb = op["rowbase"] + t * 128
                            nc.sync.dma_start(
                                out=outg[rb : rb + 128, :], in_=acc
                            )
                    if PMAJ:
                        nc.sync.dma_start(
                            out=outg[rb0 : rb0 + nt * 128, :].rearrange(
                                "(p t) d -> p t d", t=nt
                            ),
                            in_=Os[:, :nt, :],
                        )
    nc.finalize()
    return nc


# --------------------------------------------------------------- host prep --


def _wrap_ops(flat_lists):
    """Concat per-op flat index lists into the [128, cols] int16 idx tile.
    dma_gather consumes list l with out[p, j, :] = tbl[l[j*128+p]]; element i
    of each op's list lives at [i % 16, i // 16] of its column block,
    replicated x8 across partition groups (one per Q7 core)."""
    blocks = []
    for L in flat_lists:
        L = np.asarray(L, dtype=np.int16)
        assert len(L) % 16 == 0
        blocks.append(L.reshape(-1, 16).T)
    w = np.concatenate(blocks, axis=1)
    return np.ascontiguousarray(np.tile(w, (8, 1)))


def _sparse_host(inputs, title_emb_mat, nbr_batch, nbr_job, nbr_title, nbr_mask):
    """Bucket/compact per core. Returns None if no valid slots exist anywhere
    (output is then pure host assembly), else
    (plan, in_maps, row_focal per core, emb)."""
    inputs = np.asarray(inputs, dtype=np.float32)
    title_emb_mat = np.asarray(title_emb_mat, dtype=np.float32)
    emb = np.ascontiguousarray(inputs.reshape(B * P, D))
    emb16 = emb.astype(NPBF16)
    jidx = (
        np.asarray(nbr_batch, dtype=np.int64) * P + np.asarray(nbr_job, dtype=np.int64)
    ).reshape(B * P, K)
    tidx = np.asarray(nbr_title, dtype=np.int64).reshape(B * P, K)
    m = np.asarray(nbr_mask, dtype=np.int64).reshape(B * P, K)

    percore = []
    for c in range(NCORES):
        rows = slice(c * R, (c + 1) * R)
        mrow = m[rows]
        v = mrow.sum(1)
        # valid slots first, ascending k among valid (keeps the fp reduce
        # order equal to the reference: adding exact zeros is an fp no-op)
        order = np.argsort(-mrow, axis=1, kind="stable")
        js = np.take_along_axis(jidx[rows], order, 1)
        tits = np.take_along_axis(tidx[rows], order, 1)
        valid_t = tits[mrow.astype(bool)[np.arange(R)[:, None], order]]
        uniq = np.unique(valid_t)
        lut = np.zeros(T, dtype=np.int64)
        lut[uniq] = np.arange(len(uniq))
        percore.append(dict(v=v, js=js, tl=lut[tits], uniq=uniq))

    nw = {
        W: max(int((pc["v"] == W).sum()) for pc in percore) for W in WIDTHS
    }
    tiles_w = {W: -(-nw[W] // 128) for W in WIDTHS if nw[W] > 0}
    if not tiles_w:
        return None
    u_pad = max(512, -(-max(len(pc["uniq"]) for pc in percore) // 512) * 512)
    ops, idx_cols, mask_cols, total_rows = _layout(tiles_w)

    in_maps = []
    row_focal_all = []
    for c in range(NCORES):
        pc = percore[c]
        v, js, tl = pc["v"], pc["js"], pc["tl"]
        sel_w = {}
        for W in tiles_w:
            sel = np.where(v == W)[0]
            pad = tiles_w[W] * 128 - len(sel)
            sel_w[W] = np.concatenate([sel, np.full(pad, -1, dtype=np.int64)])
        flat_j = []
        flat_t = []
        flat_inv = []  # per-op: True where the slot is masked/padding
        mfs = np.zeros((128, mask_cols), dtype=np.float32)
        mbs = np.full((128, mask_cols), -1e9, dtype=np.float32)
        row_focal = np.full(total_rows, -1, dtype=np.int64)
        focal_perm = np.zeros(total_rows, dtype=np.int64)
        used_w = {W: 0 for W in sel_w}
        for op in ops:
            W, nt = op["W"], op["nt"]
            sel = sel_w[W][used_w[W] : used_w[W] + nt * 128]
            used_w[W] += nt * 128
            focs = np.where(sel < 0, 0, sel)  # dummy focal -> row 0
            vv = np.where(sel < 0, 0, v[sel])  # dummy -> fully masked
            if CONFIG["pmajor"]:
                # device row r in this op block maps to (tile r%nt, part r//nt)
                r = np.arange(nt * 128)
                src = (r % nt) * 128 + r // nt
            else:
                src = np.arange(nt * 128)
            focal_perm[op["rowbase"] : op["rowbase"] + nt * 128] = focs[src]
            row_focal[op["rowbase"] : op["rowbase"] + nt * 128] = sel[src]
            jmat = np.where(np.arange(W)[None, :] < vv[:, None], js[focs, :W], 0)
            tmat = np.where(np.arange(W)[None, :] < vv[:, None], tl[focs, :W], 0)

            def mk(mat):
                return mat.reshape(nt, 128, W).transpose(0, 2, 1).reshape(-1)

            flat_j.append(mk(jmat))
            flat_t.append(mk(tmat))
            flat_inv.append(mk(np.arange(W)[None, :] >= vv[:, None]))
            mvalid = (np.arange(W)[None, :] < vv[:, None]).reshape(nt, 128, W)
            for t in range(nt):
                mcol = op["mcol"] + t * W
                mfs[:, mcol : mcol + W] = mvalid[t].astype(np.float32)
                mbs[:, mcol : mcol + W] = (mvalid[t].astype(np.float32) - 1.0) * 1e9
        row_focal_all.append(row_focal)

        if CONFIG["sorted"]:
            # relabel both gather tables in first-access order so the random
            # row reads walk HBM mostly-ascending (page-hit friendly)
            def relabel(flats, pad_rows):
                allv = np.concatenate(flats)
                uniq_v, first = np.unique(allv, return_index=True)
                order = np.argsort(first)
                uniq_acc = uniq_v[order]  # table rows in first-access order
                lut2 = np.zeros(int(allv.max()) + 1, dtype=np.int64)
                lut2[uniq_acc] = np.arange(len(uniq_acc))
                return [lut2[f] for f in flats], uniq_acc

            flat_j, uniq_j = relabel(flat_j, None)
            flat_t, uniq_t = relabel(flat_t, None)
            emb_tbl = emb16[uniq_j]
            tloc_src = pc["uniq"][uniq_t]  # global title ids, access order
        else:
            uniq_j = None
            emb_tbl = emb16
            tloc_src = pc["uniq"]

        if CONFIG["padspread"]:
            # masked slots gather *something*; pointing them all at row 0
            # funnels one HBM region — spread them over the table instead
            jsz = len(uniq_j) if uniq_j is not None else B * P
            tsz = len(tloc_src)
            for i, inv in enumerate(flat_inv):
                n = len(inv)
                spread = (np.arange(n, dtype=np.int64) * 97)
                flat_j[i] = np.where(inv, spread % jsz, flat_j[i])
                flat_t[i] = np.where(inv, spread % tsz, flat_t[i])
        flat_lists = [x for pair in zip(flat_j, flat_t) for x in pair]
        if CONFIG["idx0"]:  # DEBUG: same-row gathers, no HBM randomness
            flat_lists = [np.zeros_like(x) for x in flat_lists]
        tloc = np.zeros((u_pad, D), dtype=NPBF16)
        tloc[: len(tloc_src)] = title_emb_mat[tloc_src].astype(NPBF16)
        in_maps.append(
            {
                "emb": emb_tbl,
                "title": tloc,
                "focal": np.ascontiguousarray(emb16[c * R + focal_perm]),
                "gidx16": _wrap_ops(flat_lists),
                "maskf": mfs,
                "maskb": mbs,
            }
        )
    if CONFIG["sorted"]:
        j_pad = -(-max(m["emb"].shape[0] for m in in_maps) // 512) * 512
        for m_ in in_maps:
            pad = np.zeros((j_pad - m_["emb"].shape[0], D), dtype=NPBF16)
            m_["emb"] = np.ascontiguousarray(np.concatenate([m_["emb"], pad], 0))
    else:
        j_pad = B * P
    plan = (tuple(sorted(tiles_w.items())), u_pad, _cfg_key(), j_pad)
    return plan, in_maps, row_focal_all, emb


# ------------------------------------------------------------------ runner --

_RUNNERS = {}


class _Runner:
    """Caches the sharded jit executable for one program variant so repeated
    executions skip retracing/recompiling (adapted from
    concourse.bass2jax.run_bass_via_pjrt's multi-core branch)."""

    def __init__(self, plan, niter):
        import jax
        from jax.experimental.shard_map import shard_map
        from jax.sharding import Mesh, NamedSharding, PartitionSpec

        from concourse import mybir as _mb
        from concourse.bass2jax import (
            _bass_exec_p,
            install_neuronx_cc_hook,
            partition_id_tensor,
        )

        install_neuronx_cc_hook()
        self.jax = jax
        nc = _build_program(plan, niter)
        self.nc = nc

        in_names, out_names, out_avals = [], [], []
        partition_name = nc.partition_id_tensor.name if nc.partition_id_tensor else None
        for alloc in nc.m.functions[0].allocations:
            if not isinstance(alloc, _mb.MemoryLocationSet):
                continue
            name = alloc.memorylocations[0].name
            if alloc.kind == "ExternalInput":
                if name != partition_name:
                    in_names.append(name)
            elif alloc.kind == "ExternalOutput":
                out_names.append(name)
                out_avals.append(
                    jax.core.ShapedArray(
                        tuple(alloc.tensor_shape), _mb.dt.np(alloc.dtype)
                    )
                )

        self.in_names = in_names
        self.out_names = out_names
        self.out_avals = out_avals
        n_params = len(in_names)
        n_outs = len(out_avals)

        bind_in_names = list(in_names) + list(out_names)
        if partition_name is not None:
            bind_in_names.append(partition_name)

        def _body(*args):
            operands = list(args)
            if partition_name is not None:
                operands.append(partition_id_tensor())
            outs = _bass_exec_p.bind(
                *operands,
                out_avals=tuple(out_avals),
                in_names=tuple(bind_in_names),
                out_names=tuple(out_names),
                lowering_input_output_aliases=(),
                sim_require_finite=True,
                sim_require_nnan=True,
                nc=nc,
            )
            return tuple(outs)

        devices = jax.devices()[:NCORES]
        mesh = Mesh(np.asarray(devices), ("core",))
        self.sharding = NamedSharding(mesh, PartitionSpec("core"))
        in_specs = (PartitionSpec("core"),) * (n_params + n_outs)
        out_specs = (PartitionSpec("core"),) * n_outs
        donate = tuple(range(n_params, n_params + n_outs))
        self.fn = jax.jit(
            shard_map(
                _body,
                mesh=mesh,
                in_specs=in_specs,
                out_specs=out_specs,
                check_rep=False,
            ),
            donate_argnums=donate,
            keep_unused=True,
        )

    def place_inputs(self, in_maps):
        concat = [
            np.concatenate([np.asarray(m[name]) for m in in_maps], axis=0)
            for name in self.in_names
        ]
        return [self.jax.device_put(a, self.sharding) for a in concat]

    def make_zeros(self):
        return [
            self.jax.device_put(
                np.zeros((NCORES * av.shape[0], *av.shape[1:]), av.dtype),
                self.sharding,
            )
            for av in self.out_avals
        ]

    def run(self, dev_in, zeros):
        return self.fn(*dev_in, *zeros)


def _get_runner(plan, niter=1):
    key = (plan, niter)
    if key not in _RUNNERS:
        _RUNNERS[key] = _Runner(plan, niter)
    return _RUNNERS[key]


# -------------------------------------------------------------- public API --


def kernel(inputs, title_emb_mat, nbr_batch, nbr_job, nbr_title, nbr_mask):
    inputs = np.asarray(inputs, dtype=np.float32)
    emb = np.ascontiguousarray(inputs.reshape(B * P, D))
    prep = _sparse_host(
        inputs, title_emb_mat, nbr_batch, nbr_job, nbr_title, nbr_mask
    )
    out = np.zeros((B * P, 2 * D), dtype=np.float32)
    out[:, :D] = emb  # focal half of the concat is literally the input rows
    if prep is None:  # every slot masked: graph half is exactly zero
        return out
    plan, in_maps, row_focal_all, _ = prep

    runner = _get_runner(plan, 1)
    dev_in = runner.place_inputs(in_maps)
    outs = runner.run(dev_in, runner.make_zeros())
    outg_full = np.asarray(outs[runner.out_names.index("outg")]).astype(np.float32)
    total_rows = outg_full.shape[0] // NCORES
    for c in range(NCORES):
        outg = outg_full[c * total_rows : (c + 1) * total_rows]
        rf = row_focal_all[c]
        valid = rf >= 0
        out[c * R + rf[valid], D:] = outg[valid]
    return out


def bench(in_maps, plan, niters=(65, 257), reps=8):
    """Per-pass device time via on-device For_i iteration scaling; min-stat
    over reps cancels most of the axon RPC jitter."""
    results = {}
    for ni in niters:
        runner = _get_runner(plan, ni)
        dev_in = runner.place_inputs(in_maps)
        zeros = [runner.make_zeros() for _ in range(reps + 1)]
        out = runner.run(dev_in, zeros[0])
        for o in out:
            o.block_until_ready()
        ts = []
        for r in range(reps):
            t0 = time.perf_counter()
            outs = runner.run(dev_in, zeros[r + 1])
            for o in outs:
                o.block_until_ready()
            ts.append(time.perf_counter() - t0)
        results[ni] = min(ts)
        print(
            f"  niter={ni}: min {min(ts) * 1e3:.3f} ms  "
            f"(all: {', '.join(f'{t * 1e3:.2f}' for t in sorted(ts))})",
            flush=True,
        )
    ni_lo, ni_hi = min(niters), max(niters)
    per_pass = (results[ni_hi] - results[ni_lo]) / (ni_hi - ni_lo)
    return per_pass * 1e9, results


# revision 28
# speedup vs baseline: 1.8544x; 1.0289x over previous
"""HAN metapath-attention kernel for 8 Trainium2 NeuronCores (Bass/Tile).

Reference computation (B=512, P=64, K=8, D=512, T=50000):
    ref_embs = inputs[nbr_batch, nbr_job]            # [B,P,K,D] gather
    raw_s    = einsum('bpd,bpkd->bpk', inputs, ref_embs)
    sim      = softmax(where(mask, raw_s, -1e9)) * mask
    out      = concat([inputs, einsum('bpk,bpkt->bpt', sim, title[nbr_title])], -1)
    returns [B*P, 2D] f32

Sharding: data-parallel over flat rows r = b*P + p; core c owns rows
[c*4096, (c+1)*4096). The flattened `inputs` gather table is replicated to
every core's HBM; `title_emb_mat` is compacted per core (only referenced
rows; always <= 32767 distinct so local ids fit int16). No cross-core
traffic is needed.

Sparsity: mask ~ Bernoulli(1/2) makes half the neighbor slots contribute
EXACTLY zero, so the kernel only gathers valid slots. Focals are bucketed
by exact valid-count v = W in {1..8}; each 128-focal tile is uniform-W so
no per-slot masking waste remains (only <128 round-up pad focals per
bucket). v=0 focals (graph half exactly 0) never touch the device.

Precision: all embedding tables (focal rows, neighbor-job rows, title
rows) are bf16 on device, halving the random-row gather traffic that
bounds this kernel. Dot-product logits accumulate in f32 (stt accum_out);
softmax runs in f32; the weighted title accumulation and output store are
bf16. Emulated end-to-end rel-err vs the f32 reference is 5.8e-3, within
the 2e-2 gate.

Device work per 128-focal tile: one batched int16 dma_gather each for the
neighbor-job rows (SWDGE queue 0) and title rows (queue 1), superblocked
<= 1024 indices/op, a sequential HWDGE load of the (host pre-permuted)
focal rows, fused bf16 product+f32-row-reduce for the dots, a free-dim
masked softmax, a fused bf16 mult+add chain for the weighted title sum,
and one [128,512] bf16 store of the graph half. The focal half of the
output is host-assembled (it is literally the input rows); device output
rows are unpermuted on the host.
"""

import sys
import time

if "/opt/trn_rl_repo" not in sys.path:
    sys.path.insert(0, "/opt/trn_rl_repo")

import numpy as np
import ml_dtypes

import concourse.bacc as bacc
import concourse.bass as bass
import concourse.tile as tile
from concourse import mybir

B, P, K, D, T = 512, 64, 8, 512, 50000
NCORES = 8
R = B * P // NCORES  # 4096 focal rows per core
F32 = mybir.dt.float32
BF16 = mybir.dt.bfloat16
I16 = mybir.dt.int16
NPBF16 = ml_dtypes.bfloat16
WIDTHS = (1, 2, 3, 4, 5, 6, 7, 8)

# Tunables (A/B swept on hardware; defaults = best measured combination).
CONFIG = dict(
    single_packet=False,  # dma_gather packetization mode
    nqueues=2,  # SWDGE queues: J on even, T on odd, alternating per op
    pmajor=False,  # p-major focal/out rows: 1 contiguous desc per partition
    sbmax=8,  # max slots (nt*W) per gather superblock
    gbufs=6,  # gather tile pool buffering (deep: keeps SDMA queues fed)
    sorted=True,  # relabel gather tables in first-access order (HBM locality)
    fbufs=5,  # focal load pool buffering
    nocompute=False,  # DEBUG: replace attention compute by one add (DMA floor)
    idx0=False,  # DEBUG: all gather indices -> row 0 (no HBM randomness)
    padspread=True,  # spread masked-slot gather indices across the table
    cbufs=True,  # deeper compute pools (wp=4, sp=6)
)


def _cfg_key():
    return tuple(sorted(CONFIG.items()))


# ------------------------------------------------------------------ layout --


def _layout(tiles_w):
    """Emission-order op layout shared by builder and host prep.

    Each op is one superblock: nt tiles of width W, with one J gather and one
    T gather (128*nt*W indices each), a sequential focal load, nt compute
    tiles, and nt output stores. Returns (ops, idx_cols, mask_cols, rows).
    """
    ops = []
    col = 0
    mcol = 0
    rowbase = 0
    for W in WIDTHS:
        SB = max(1, CONFIG["sbmax"] // W)
        t = 0
        while t < tiles_w.get(W, 0):
            nt = min(SB, tiles_w[W] - t)
            jn = 128 * nt * W
            op = dict(W=W, nt=nt, jcol=col, mcol=mcol, rowbase=rowbase)
            col += jn // 16
            op["tcol"] = col
            col += jn // 16
            ops.append(op)
            mcol += nt * W
            rowbase += nt * 128
            t += nt
    return ops, col, mcol, rowbase


# ----------------------------------------------------------------- builder --


def _build_program(plan, niter=1):
    """plan = (tiles_w item tuple, u_pad). niter>1 wraps the pass in a For_i
    loop (bench-only; makes device time dominate one execution)."""
    tiles_w = dict(plan[0])
    u_pad = plan[1]
    assert plan[2] == _cfg_key(), "plan built under a different CONFIG"
    ops, idx_cols, mask_cols, total_rows = _layout(tiles_w)
    assert total_rows > 0
    SBM = CONFIG["sbmax"]
    NQ = CONFIG["nqueues"]
    SP_ = CONFIG["single_packet"]
    PMAJ = CONFIG["pmajor"]

    nc = bacc.Bacc(
        "TRN2", target_bir_lowering=False, debug=False,
        num_swdge_queues=max(2, NQ),
    )
    j_rows = plan[3] if len(plan) > 3 else B * P
    emb = nc.dram_tensor("emb", [j_rows, D], BF16, kind="ExternalInput")
    title = nc.dram_tensor("title", [u_pad, D], BF16, kind="ExternalInput")
    # bucket-ordered focal rows (host pre-permuted): row rb + t*128 + p is the
    # focal embedding of (tile t, partition p) -> sequential HWDGE loads
    focal = nc.dram_tensor("focal", [total_rows, D], BF16, kind="ExternalInput")
    gidx16 = nc.dram_tensor("gidx16", [128, idx_cols], I16, kind="ExternalInput")
    maskf = nc.dram_tensor("maskf", [128, mask_cols], F32, kind="ExternalInput")
    maskb = nc.dram_tensor("maskb", [128, mask_cols], F32, kind="ExternalInput")
    outg = nc.dram_tensor("outg", [total_rows, D], BF16, kind="ExternalOutput")

    with tile.TileContext(nc) as tc:
        GB = CONFIG["gbufs"]
        with (
            tc.tile_pool(name="idxp", bufs=1) as idxp,
            tc.tile_pool(name="fp", bufs=CONFIG["fbufs"]) as fp,
            tc.tile_pool(name="jp", bufs=GB) as jp,
            tc.tile_pool(name="tp", bufs=GB) as tp,
            tc.tile_pool(name="wp", bufs=4 if CONFIG["cbufs"] else 3) as wp,
            tc.tile_pool(name="op_", bufs=2) as op_,
            tc.tile_pool(name="sp", bufs=6 if CONFIG["cbufs"] else 4) as sp,
        ):
            gx = idxp.tile([128, idx_cols], I16)
            mf = idxp.tile([128, mask_cols], F32)
            mb = idxp.tile([128, mask_cols], F32)
            nc.sync.dma_start(out=gx[:], in_=gidx16[:])
            nc.sync.dma_start(out=mf[:], in_=maskf[:])
            nc.sync.dma_start(out=mb[:], in_=maskb[:])

            import contextlib

            loop_ctx = (
                tc.For_i(0, niter, 1) if niter > 1 else contextlib.nullcontext()
            )
            with loop_ctx:
                for iop, op in enumerate(ops):
                    W, nt = op["W"], op["nt"]
                    jn = 128 * nt * W
                    rb0 = op["rowbase"]
                    Fs = fp.tile([128, SBM, D], BF16, tag="F")
                    if PMAJ:
                        fsrc = focal[rb0 : rb0 + nt * 128, :].rearrange(
                            "(p t) d -> p t d", t=nt
                        )
                    else:
                        fsrc = focal[rb0 : rb0 + nt * 128, :].rearrange(
                            "(t p) d -> p t d", p=128
                        )
                    nc.sync.dma_start(out=Fs[:, :nt, :], in_=fsrc)
                    jq = (2 * iop) % NQ
                    tq = (2 * iop + 1) % NQ
                    Js = jp.tile([128, SBM, D], BF16, tag="J")
                    nc.gpsimd.dma_gather(
                        Js[:, : nt * W, :],
                        emb[:],
                        gx[:, op["jcol"] : op["jcol"] + jn // 16],
                        jn,
                        jn,
                        D,
                        queue_num=jq,
                        single_packet=SP_,
                    )
                    Ts = tp.tile([128, SBM, D], BF16, tag="T")
                    nc.gpsimd.dma_gather(
                        Ts[:, : nt * W, :],
                        title[:],
                        gx[:, op["tcol"] : op["tcol"] + jn // 16],
                        jn,
                        jn,
                        D,
                        queue_num=tq,
                        single_packet=SP_,
                    )
                    Os = (
                        op_.tile([128, SBM, D], BF16, tag="Os", name="Os")
                        if PMAJ
                        else None
                    )
                    for t in range(nt):
                        mc = op["mcol"] + t * W
                        if PMAJ:
                            acc = Os[:, t, :]
                        else:
                            acc_t = wp.tile([128, D], BF16, tag="acc", name="acc")
                            acc = acc_t[:]
                        if CONFIG["nocompute"]:
                            # keep all three loads live via two adds, no attention
                            nc.vector.scalar_tensor_tensor(
                                out=acc,
                                in0=Fs[:, t, :],
                                scalar=1.0,
                                in1=Js[:, t * W, :],
                                op0=mybir.AluOpType.mult,
                                op1=mybir.AluOpType.add,
                            )
                            nc.vector.tensor_tensor(
                                out=acc,
                                in0=acc,
                                in1=Ts[:, t * W, :],
                                op=mybir.AluOpType.add,
                            )
                            if not PMAJ:
                                rb = op["rowbase"] + t * 128
                                nc.sync.dma_start(
                                    out=outg[rb : rb + 128, :], in_=acc
                                )
                            continue
                        if W == 1:
                            # single valid slot: sim == mask (1 valid, 0 pad)
                            nc.vector.tensor_scalar_mul(
                                out=acc,
                                in0=Ts[:, t, :],
                                scalar1=mf[:, mc : mc + 1],
                            )
                            if not PMAJ:
                                rb = op["rowbase"] + t * 128
                                nc.sync.dma_start(
                                    out=outg[rb : rb + 128, :], in_=acc
                                )
                            continue
                        # dots[:, k] = sum_d F * J_k (fused product+row-reduce)
                        dots = sp.tile([128, 8], F32, tag="dots")
                        prod = wp.tile([128, D], BF16, tag="prod")
                        for k in range(W):
                            nc.vector.scalar_tensor_tensor(
                                out=prod[:],
                                in0=Fs[:, t, :],
                                scalar=1.0,
                                in1=Js[:, t * W + k, :],
                                op0=mybir.AluOpType.mult,
                                op1=mybir.AluOpType.mult,
                                accum_out=dots[:, k : k + 1],
                            )
                        # masked logits = dots + (0 | -1e9)
                        logits = sp.tile([128, 8], F32, tag="logits")
                        nc.vector.tensor_tensor(
                            out=logits[:, :W],
                            in0=dots[:, :W],
                            in1=mb[:, mc : mc + W],
                            op=mybir.AluOpType.add,
                        )
                        negM = sp.tile([128, 1], F32, tag="negM")
                        nc.vector.tensor_reduce(
                            out=negM[:],
                            in_=logits[:, :W],
                            axis=mybir.AxisListType.X,
                            op=mybir.AluOpType.max,
                            negate=True,
                        )
                        e = sp.tile([128, 8], F32, tag="e")
                        nc.scalar.activation(
                            out=e[:, :W],
                            in_=logits[:, :W],
                            func=mybir.ActivationFunctionType.Exp,
                            bias=negM[:, 0:1],
                            scale=1.0,
                        )
                        ssum = sp.tile([128, 1], F32, tag="ssum")
                        nc.vector.tensor_reduce(
                            out=ssum[:],
                            in_=e[:, :W],
                            axis=mybir.AxisListType.X,
                            op=mybir.AluOpType.add,
                        )
                        rr = sp.tile([128, 1], F32, tag="rr")
                        nc.vector.reciprocal(out=rr[:], in_=ssum[:])
                        sim = sp.tile([128, 8], F32, tag="sim")
                        nc.vector.scalar_tensor_tensor(
                            out=sim[:, :W],
                            in0=e[:, :W],
                            scalar=rr[:, 0:1],
                            in1=mf[:, mc : mc + W],
                            op0=mybir.AluOpType.mult,
                            op1=mybir.AluOpType.mult,
                        )
                        # weighted title sum (fused mult+add chain)
                        nc.vector.tensor_scalar_mul(
                            out=acc, in0=Ts[:, t * W, :], scalar1=sim[:, 0:1]
                        )
                        for k in range(1, W):
                            nc.vector.scalar_tensor_tensor(
                                out=acc,
                                in0=Ts[:, t * W + k, :],
                                scalar=sim[:, k : k + 1],
                                in1=acc,
                                op0=mybir.AluOpType.mult,
                                op1=mybir.AluOpType.add,
                            )
                        if not PMAJ:
                            rb = op["rowbase"] + t * 128
                            nc.sync.dma_start(
                                out=outg[rb : rb + 128, :], in_=acc
                            )
                    if PMAJ:
                        nc.sync.dma_start(
                            out=outg[rb0 : rb0 + nt * 128, :].rearrange(
                                "(p t) d -> p t d", t=nt
                            ),
                            in_=Os[:, :nt, :],
                        )
    nc.finalize()
    return nc


# --------------------------------------------------------------- host prep --


def _wrap_ops(flat_lists):
    """Concat per-op flat index lists into the [128, cols] int16 idx tile.
    dma_gather consumes list l with out[p, j, :] = tbl[l[j*128+p]]; element i
    of each op's list lives at [i % 16, i // 16] of its column block,
    replicated x8 across partition groups (one per Q7 core)."""
    blocks = []
    for L in flat_lists:
        L = np.asarray(L, dtype=np.int16)
        assert len(L) % 16 == 0
        blocks.append(L.reshape(-1, 16).T)
    w = np.concatenate(blocks, axis=1)
    return np.ascontiguousarray(np.tile(w, (8, 1)))


def _sparse_host(inputs, title_emb_mat, nbr_batch, nbr_job, nbr_title, nbr_mask):
    """Bucket/compact per core. Returns None if no valid slots exist anywhere
    (output is then pure host assembly), else
    (plan, in_maps, row_focal per core, emb)."""
    inputs = np.asarray(inputs, dtype=np.float32)
    title_emb_mat = np.asarray(title_emb_mat, dtype=np.float32)
    emb = np.ascontiguousarray(inputs.reshape(B * P, D))
    emb16 = emb.astype(NPBF16)
    jidx = (
        np.asarray(nbr_batch, dtype=np.int64) * P + np.asarray(nbr_job, dtype=np.int64)
    ).reshape(B * P, K)
    tidx = np.asarray(nbr_title, dtype=np.int64).reshape(B * P, K)
    m = np.asarray(nbr_mask, dtype=np.int64).reshape(B * P, K)

    percore = []
    for c in range(NCORES):
        rows = slice(c * R, (c + 1) * R)
        mrow = m[rows]
        v = mrow.sum(1)
        # valid slots first, ascending k among valid (keeps the fp reduce
        # order equal to the reference: adding exact zeros is an fp no-op)
        order = np.argsort(-mrow, axis=1, kind="stable")
        js = np.take_along_axis(jidx[rows], order, 1)
        tits = np.take_along_axis(tidx[rows], order, 1)
        valid_t = tits[mrow.astype(bool)[np.arange(R)[:, None], order]]
        uniq = np.unique(valid_t)
        lut = np.zeros(T, dtype=np.int64)
        lut[uniq] = np.arange(len(uniq))
        percore.append(dict(v=v, js=js, tl=lut[tits], uniq=uniq))

    nw = {
        W: max(int((pc["v"] == W).sum()) for pc in percore) for W in WIDTHS
    }
    tiles_w = {W: -(-nw[W] // 128) for W in WIDTHS if nw[W] > 0}
    if not tiles_w:
        return None
    u_pad = max(512, -(-max(len(pc["uniq"]) for pc in percore) // 512) * 512)
    ops, idx_cols, mask_cols, total_rows = _layout(tiles_w)

    in_maps = []
    row_focal_all = []
    for c in range(NCORES):
        pc = percore[c]
        v, js, tl = pc["v"], pc["js"], pc["tl"]
        sel_w = {}
        for W in tiles_w:
            sel = np.where(v == W)[0]
            pad = tiles_w[W] * 128 - len(sel)
            sel_w[W] = np.concatenate([sel, np.full(pad, -1, dtype=np.int64)])
        flat_j = []
        flat_t = []
        flat_inv = []  # per-op: True where the slot is masked/padding
        mfs = np.zeros((128, mask_cols), dtype=np.float32)
        mbs = np.full((128, mask_cols), -1e9, dtype=np.float32)
        row_focal = np.full(total_rows, -1, dtype=np.int64)
        focal_perm = np.zeros(total_rows, dtype=np.int64)
        used_w = {W: 0 for W in sel_w}
        for op in ops:
            W, nt = op["W"], op["nt"]
            sel = sel_w[W][used_w[W] : used_w[W] + nt * 128]
            used_w[W] += nt * 128
            focs = np.where(sel < 0, 0, sel)  # dummy focal -> row 0
            vv = np.where(sel < 0, 0, v[sel])  # dummy -> fully masked
            if CONFIG["pmajor"]:
                # device row r in this op block maps to (tile r%nt, part r//nt)
                r = np.arange(nt * 128)
                src = (r % nt) * 128 + r // nt
            else:
                src = np.arange(nt * 128)
            focal_perm[op["rowbase"] : op["rowbase"] + nt * 128] = focs[src]
            row_focal[op["rowbase"] : op["rowbase"] + nt * 128] = sel[src]
            jmat = np.where(np.arange(W)[None, :] < vv[:, None], js[focs, :W], 0)
            tmat = np.where(np.arange(W)[None, :] < vv[:, None], tl[focs, :W], 0)

            def mk(mat):
                return mat.reshape(nt, 128, W).transpose(0, 2, 1).reshape(-1)

            flat_j.append(mk(jmat))
            flat_t.append(mk(tmat))
            flat_inv.append(mk(np.arange(W)[None, :] >= vv[:, None]))
            mvalid = (np.arange(W)[None, :] < vv[:, None]).reshape(nt, 128, W)
            for t in range(nt):
                mcol = op["mcol"] + t * W
                mfs[:, mcol : mcol + W] = mvalid[t].astype(np.float32)
                mbs[:, mcol : mcol + W] = (mvalid[t].astype(np.float32) - 1.0) * 1e9
        row_focal_all.append(row_focal)

        if CONFIG["sorted"]:
            # relabel both gather tables in first-access order so the random
            # row reads walk HBM mostly-ascending (page-hit friendly)
            def relabel(flats, pad_rows):
                allv = np.concatenate(flats)
                uniq_v, first = np.unique(allv, return_index=True)
                order = np.argsort(first)
                uniq_acc = uniq_v[order]  # table rows in first-access order
                lut2 = np.zeros(int(allv.max()) + 1, dtype=np.int64)
                lut2[uniq_acc] = np.arange(len(uniq_acc))
                return [lut2[f] for f in flats], uniq_acc

            flat_j, uniq_j = relabel(flat_j, None)
            flat_t, uniq_t = relabel(flat_t, None)
            emb_tbl = emb16[uniq_j]
            tloc_src = pc["uniq"][uniq_t]  # global title ids, access order
        else:
            uniq_j = None
            emb_tbl = emb16
            tloc_src = pc["uniq"]

        if CONFIG["padspread"]:
            # masked slots gather *something*; pointing them all at row 0
            # funnels one HBM region — spread them over the table instead
            jsz = len(uniq_j) if uniq_j is not None else B * P
            tsz = len(tloc_src)
            for i, inv in enumerate(flat_inv):
                n = len(inv)
                spread = (np.arange(n, dtype=np.int64) * 97)
                flat_j[i] = np.where(inv, spread % jsz, flat_j[i])
                flat_t[i] = np.where(inv, spread % tsz, flat_t[i])
        flat_lists = [x for pair in zip(flat_j, flat_t) for x in pair]
        if CONFIG["idx0"]:  # DEBUG: same-row gathers, no HBM randomness
            flat_lists = [np.zeros_like(x) for x in flat_lists]
        tloc = np.zeros((u_pad, D), dtype=NPBF16)
        tloc[: len(tloc_src)] = title_emb_mat[tloc_src].astype(NPBF16)
        in_maps.append(
            {
                "emb": emb_tbl,
                "title": tloc,
                "focal": np.ascontiguousarray(emb16[c * R + focal_perm]),
                "gidx16": _wrap_ops(flat_lists),
                "maskf": mfs,
                "maskb": mbs,
            }
        )
    if CONFIG["sorted"]:
        j_pad = -(-max(m["emb"].shape[0] for m in in_maps) // 512) * 512
        for m_ in in_maps:
            pad = np.zeros((j_pad - m_["emb"].shape[0], D), dtype=NPBF16)
            m_["emb"] = np.ascontiguousarray(np.concatenate([m_["emb"], pad], 0))
    else:
        j_pad = B * P
    plan = (tuple(sorted(tiles_w.items())), u_pad, _cfg_key(), j_pad)
    return plan, in_maps, row_focal_all, emb


# ------------------------------------------------------------------ runner --

_RUNNERS = {}


class _Runner:
    """Caches the sharded jit executable for one program variant so repeated
    executions skip retracing/recompiling (adapted from
    concourse.bass2jax.run_bass_via_pjrt's multi-core branch)."""

    def __init__(self, plan, niter):
        import jax
        from jax.experimental.shard_map import shard_map
        from jax.sharding import Mesh, NamedSharding, PartitionSpec

        from concourse import mybir as _mb
        from concourse.bass2jax import (
            _bass_exec_p,
            install_neuronx_cc_hook,
            partition_id_tensor,
        )

        install_neuronx_cc_hook()
        self.jax = jax
        nc = _build_program(plan, niter)
        self.nc = nc

        in_names, out_names, out_avals = [], [], []
        partition_name = nc.partition_id_tensor.name if nc.partition_id_tensor else None
        for alloc in nc.m.functions[0].allocations:
            if not isinstance(alloc, _mb.MemoryLocationSet):
                continue
            name = alloc.memorylocations[0].name
            if alloc.kind == "ExternalInput":
                if name != partition_name:
                    in_names.append(name)
            elif alloc.kind == "ExternalOutput":
                out_names.append(name)
                out_avals.append(
                    jax.core.ShapedArray(
                        tuple(alloc.tensor_shape), _mb.dt.np(alloc.dtype)
                    )
                )

        self.in_names = in_names
        self.out_names = out_names
        self.out_avals = out_avals
        n_params = len(in_names)
        n_outs = len(out_avals)

        bind_in_names = list(in_names) + list(out_names)
        if partition_name is not None:
            bind_in_names.append(partition_name)

        def _body(*args):
            operands = list(args)
            if partition_name is not None:
                operands.append(partition_id_tensor())
            outs = _bass_exec_p.bind(
                *operands,
                out_avals=tuple(out_avals),
                in_names=tuple(bind_in_names),
                out_names=tuple(out_names),
                lowering_input_output_aliases=(),
                sim_require_finite=True,
                sim_require_nnan=True,
                nc=nc,
            )
            return tuple(outs)

        devices = jax.devices()[:NCORES]
        mesh = Mesh(np.asarray(devices), ("core",))
        self.sharding = NamedSharding(mesh, PartitionSpec("core"))
        in_specs = (PartitionSpec("core"),) * (n_params + n_outs)
        out_specs = (PartitionSpec("core"),) * n_outs
        donate = tuple(range(n_params, n_params + n_outs))
        self.fn = jax.jit(
            shard_map(
                _body,
                mesh=mesh,
                in_specs=in_specs,
                out_specs=out_specs,
                check_rep=False,
            ),
            donate_argnums=donate,
            keep_unused=True,
        )

    def place_inputs(self, in_maps):
        concat = [
            np.concatenate([np.asarray(m[name]) for m in in_maps], axis=0)
            for name in self.in_names
        ]
        return [self.jax.device_put(a, self.sharding) for a in concat]

    def make_zeros(self):
        return [
            self.jax.device_put(
                np.zeros((NCORES * av.shape[0], *av.shape[1:]), av.dtype),
                self.sharding,
            )
            for av in self.out_avals
        ]

    def run(self, dev_in, zeros):
        return self.fn(*dev_in, *zeros)


def _get_runner(plan, niter=1):
    key = (plan, niter)
    if key not in _RUNNERS:
        _RUNNERS[key] = _Runner(plan, niter)
    return _RUNNERS[key]


# -------------------------------------------------------------- public API --


def kernel(inputs, title_emb_mat, nbr_batch, nbr_job, nbr_title, nbr_mask):
    inputs = np.asarray(inputs, dtype=np.float32)
    emb = np.ascontiguousarray(inputs.reshape(B * P, D))
    prep = _sparse_host(
        inputs, title_emb_mat, nbr_batch, nbr_job, nbr_title, nbr_mask
    )
    out = np.zeros((B * P, 2 * D), dtype=np.float32)
    out[:, :D] = emb  # focal half of the concat is literally the input rows
    if prep is None:  # every slot masked: graph half is exactly zero
        return out
    plan, in_maps, row_focal_all, _ = prep

    runner = _get_runner(plan, 1)
    dev_in = runner.place_inputs(in_maps)
    outs = runner.run(dev_in, runner.make_zeros())
    outg_full = np.asarray(outs[runner.out_names.index("outg")]).astype(np.float32)
    total_rows = outg_full.shape[0] // NCORES
    for c in range(NCORES):
        outg = outg_full[c * total_rows : (c + 1) * total_rows]
        rf = row_focal_all[c]
        valid = rf >= 0
        out[c * R + rf[valid], D:] = outg[valid]
    return out


def bench(in_maps, plan, niters=(65, 257), reps=8):
    """Per-pass device time via on-device For_i iteration scaling; min-stat
    over reps cancels most of the axon RPC jitter."""
    results = {}
    for ni in niters:
        runner = _get_runner(plan, ni)
        dev_in = runner.place_inputs(in_maps)
        zeros = [runner.make_zeros() for _ in range(reps + 1)]
        out = runner.run(dev_in, zeros[0])
        for o in out:
            o.block_until_ready()
        ts = []
        for r in range(reps):
            t0 = time.perf_counter()
            outs = runner.run(dev_in, zeros[r + 1])
            for o in outs:
                o.block_until_ready()
            ts.append(time.perf_counter() - t0)
        results[ni] = min(ts)
        print(
            f"  niter={ni}: min {min(ts) * 1e3:.3f} ms  "
            f"(all: {', '.join(f'{t * 1e3:.2f}' for t in sorted(ts))})",
            flush=True,
        )
    ni_lo, ni_hi = min(niters), max(niters)
    per_pass = (results[ni_hi] - results[ni_lo]) / (ni_hi - ni_lo)
    return per_pass * 1e9, results
